# revision 5
# baseline (speedup 1.0000x reference)
"""Llama attention (N=2, S=2048, H=2048, nh=16, dh=128) on 8 NeuronCores.

v3: v1 structure, tuned against the TimelineSim cost model:
- xt staged chunk-major in DRAM so every chunk DMA is one contiguous
  2MB read; xt chunk 0 issued before the weight loads so PE starts ~6us
  earlier.
- Separate PSUM tag budgets per phase role (no cross-phase WAR chains).
- Attention inner order scores -> denominator -> context so the
  ssb/reciprocal chain on Scalar/Vector overlaps the context matmuls.
- Partial outputs in bf16: halves output DMA and eviction time; host
  accumulates in f32.
"""

import math
from functools import lru_cache

import numpy as np
import ml_dtypes

N_CORES = 8
N, S, H = 2, 2048, 2048
NH, DH = 16, 128
HPC = NH // N_CORES          # heads per core = 2
T = N * S                    # 4096 tokens
P = 128
KI = H // P                  # 16 contraction subtiles for projections
TCH = 512                    # projection token chunk
QCH = 512                    # attention q chunk
SB = S // P                  # 16 key blocks per batch
HALF = DH // 2


def _build_nc(repeat=1):
    import concourse.mybir as mybir
    import concourse.tile as tile
    from concourse import bacc

    fp32 = mybir.dt.float32
    bf16 = mybir.dt.bfloat16
    EXP = mybir.ActivationFunctionType.Exp
    COPY = mybir.ActivationFunctionType.Copy

    nc = bacc.Bacc("TRN2", target_bir_lowering=False, debug=False,
                   num_devices=N_CORES)
    # xtc[c, p, k, t] = X.T[(k*128+p), c*512 + t]   (chunk-major, contiguous)
    xtc = nc.dram_tensor("xtc", [T // TCH, P, KI, TCH], bf16,
                         kind="ExternalInput")
    wqt = nc.dram_tensor("wqt", [P, KI, HPC * DH], bf16, kind="ExternalInput")
    wkt = nc.dram_tensor("wkt", [P, KI, HPC * DH], bf16, kind="ExternalInput")
    wvt = nc.dram_tensor("wvt", [P, KI, HPC * DH], bf16, kind="ExternalInput")
    wot = nc.dram_tensor("wot", [P, HPC, H], bf16, kind="ExternalInput")
    cos2 = nc.dram_tensor("cos2", [P, S], fp32, kind="ExternalInput")
    sinp = nc.dram_tensor("sinp", [HALF, S], fp32, kind="ExternalInput")
    tril = nc.dram_tensor("tril", [P, P], bf16, kind="ExternalInput")
    out = nc.dram_tensor("out", [T, H], bf16, kind="ExternalOutput")

    inv_sqrt_dh = 1.0 / math.sqrt(DH)
    n_tch = T // TCH            # 8 projection chunks
    n_qch = S // QCH            # 4 attention q-chunks per (head, batch)

    from contextlib import ExitStack

    with tile.TileContext(nc) as tc, ExitStack() as es:
        consts = es.enter_context(tc.tile_pool(name="consts", bufs=1))
        wpool = es.enter_context(tc.tile_pool(name="wpool", bufs=1))
        xtp = es.enter_context(tc.tile_pool(name="xtp", bufs=2))
        qkv = es.enter_context(tc.tile_pool(name="qkv", bufs=1))
        wt_pool = es.enter_context(tc.tile_pool(name="wt", bufs=2))
        ctx_pool = es.enter_context(tc.tile_pool(name="ctxp", bufs=2))
        outp = es.enter_context(tc.tile_pool(name="outp", bufs=3))
        tmp = es.enter_context(tc.tile_pool(name="tmp", bufs=2))
        # PSUM: pqk 2 + pv 1 + ps 3 + sum 1 + ctx 1 = 8 banks
        ps_qk = es.enter_context(tc.tile_pool(name="ps_qk", bufs=2,
                                              space="PSUM"))
        ps_v = es.enter_context(tc.tile_pool(name="ps_v", bufs=1,
                                             space="PSUM"))
        ps_s = es.enter_context(tc.tile_pool(name="ps_s", bufs=2,
                                             space="PSUM"))
        ps_c = es.enter_context(tc.tile_pool(name="ps_c", bufs=1,
                                             space="PSUM"))

        # ---- first xt chunk DMA before anything else ----
        xt_tiles = {}
        xt_tiles[0] = xtp.tile([P, KI, TCH], bf16, tag="xt", name="xt0")
        nc.scalar.dma_start(xt_tiles[0][:, :KI // 2], xtc[0][:, :KI // 2])
        nc.scalar.dma_start(xt_tiles[0][:, KI // 2:], xtc[0][:, KI // 2:])

        # ---- constants / weights in SBUF ----
        wq_t = wpool.tile([P, KI, HPC * DH], bf16)
        nc.sync.dma_start(wq_t[:, :KI // 2], wqt[:, :KI // 2])
        nc.sync.dma_start(wq_t[:, KI // 2:], wqt[:, KI // 2:])
        wk_t = wpool.tile([P, KI, HPC * DH], bf16)
        nc.sync.dma_start(wk_t[:], wkt[:])
        wv_t = wpool.tile([P, KI, HPC * DH], bf16)
        nc.sync.dma_start(wv_t[:], wvt[:])
        cos2_t = consts.tile([P, S], fp32)
        nc.sync.dma_start(cos2_t[:], cos2[:])
        sinp_t = consts.tile([HALF, S], fp32)
        nc.sync.dma_start(sinp_t[:], sinp[:])
        ones_col = consts.tile([P, 1], bf16)
        nc.vector.memset(ones_col[:], 1.0)
        ones_row = consts.tile([1, P], bf16)
        nc.vector.memset(ones_row[:], 1.0)
        tril_t = consts.tile([P, P], bf16)
        nc.sync.dma_start(tril_t[:], tril[:])
        wo_t = wpool.tile([P, HPC, H], bf16)
        nc.sync.dma_start(wo_t[:], wot[:])

        # ---- per (head, batch) activation stores ----
        qT = [[qkv.tile([P, S], bf16, tag=f"q{h}{b}", name=f"q{h}{b}")
               for b in range(N)] for h in range(HPC)]
        kT = [[qkv.tile([P, S], bf16, tag=f"k{h}{b}", name=f"k{h}{b}")
               for b in range(N)] for h in range(HPC)]
        vS = [qkv.tile([P, SB, HPC * DH], bf16, tag=f"v{b}", name=f"v{b}")
              for b in range(N)]

        def rope_evict(ps, dst, s0):
            # dst[:, s0:s0+TCH] = bf16(RoPE(ps)); ps is [128, TCH] f32 PSUM
            ra = tmp.tile([P, TCH], fp32, tag="ropeA")
            rb = tmp.tile([P, TCH], fp32, tag="ropeB")
            cs = slice(s0, s0 + TCH)
            nc.vector.tensor_mul(ra[:], ps[:], cos2_t[:, cs])
            nc.vector.tensor_mul(rb[:HALF, :], ps[HALF:, :], sinp_t[:, cs])
            nc.vector.tensor_mul(rb[HALF:, :], ps[:HALF, :], sinp_t[:, cs])
            nc.vector.tensor_sub(dst[:HALF, cs], ra[:HALF, :], rb[:HALF, :])
            nc.vector.tensor_add(dst[HALF:, cs], ra[HALF:, :], rb[HALF:, :])

        for _rep in range(repeat):
          # ---- projections ----
          for c in range(n_tch):
            t0 = c * TCH
            b = t0 // S
            s0 = t0 - b * S
            if c not in xt_tiles:
                xt_tiles[c] = xtp.tile([P, KI, TCH], bf16, tag="xt",
                                       name=f"xt{c}")
                nc.sync.dma_start(xt_tiles[c][:], xtc[c])
            xt_t = xt_tiles[c]

            for h in range(HPC):
                d0 = h * DH
                for (wsb, dstT) in ((wq_t, qT), (wk_t, kT)):
                    ps = ps_qk.tile([P, TCH], fp32, tag="qk")
                    for k in range(KI):
                        nc.tensor.matmul(ps[:], wsb[:, k, d0:d0 + DH],
                                         xt_t[:, k, :],
                                         start=(k == 0), stop=(k == KI - 1))
                    rope_evict(ps, dstT[h][b], s0)

            # V: natural [t, d] layout, both heads at once (n = 256)
            for ts_ in range(TCH // P):
                psv = ps_v.tile([P, 512], fp32, tag="projv")
                ps = psv[:, :HPC * DH]
                for k in range(KI):
                    nc.tensor.matmul(ps[:], xt_t[:, k, ts_ * P:(ts_ + 1) * P],
                                     wv_t[:, k, :],
                                     start=(k == 0), stop=(k == KI - 1))
                blk = s0 // P + ts_
                nc.scalar.activation(vS[b][:, blk, :], ps[:], COPY)
            if c + 1 < n_tch and _rep == 0:
                pass  # next chunk DMA issued at loop top (bufs=2 overlap)
          xt_tiles.clear()

          # ---- attention + fused partial output projection ----
          for b in range(N):
              for qc in range(n_qch):
                  q0 = qc * QCH
                  nkb = (q0 + QCH) // P       # causal k-block count
                  ctxT = ctx_pool.tile([P, HPC, QCH], bf16, tag="ctx")
                  for h in range(HPC):
                      wtile = wt_pool.tile([P, SB, QCH], bf16, tag="wt")
                      for kp in range(nkb // 2):
                          ps = ps_s.tile([P, 2, QCH], fp32, tag="sc")
                          for j in range(2):
                              kb = 2 * kp + j
                              nc.tensor.matmul(
                                  ps[:, j, :],
                                  kT[h][b][:, kb * P:(kb + 1) * P],
                                  qT[h][b][:, q0:q0 + QCH],
                                  start=True, stop=True,
                                  skip_group_check=True)
                          nc.scalar.activation(
                              wtile[:, 2 * kp:2 * kp + 2, :]
                              .rearrange("p a q -> p (a q)"),
                              ps[:].rearrange("p a q -> p (a q)"),
                              EXP, scale=inv_sqrt_dh)
                      # diagonal fixups (last 4 kb blocks of this q chunk)
                      for kb in range(nkb - 4, nkb):
                          dd = kb * P - q0
                          if dd > 0:
                              nc.vector.memset(wtile[:, kb, :dd], 0.0)
                          nc.vector.tensor_mul(wtile[:, kb, dd:dd + P],
                                               wtile[:, kb, dd:dd + P],
                                               tril_t[:])
                      # softmax denominator via PE ones-matmul over k
                      spsq = ps_qk.tile([P, QCH], fp32, tag="qk")
                      sps = spsq[0:1, :]
                      for kb in range(nkb):
                          dd = max(kb * P - q0, 0)
                          nc.tensor.matmul(sps[:, dd:], ones_col[:],
                                           wtile[:, kb, dd:],
                                           start=(kb == 0),
                                           stop=(kb == nkb - 1),
                                           skip_group_check=True)
                      rsb = tmp.tile([1, QCH], bf16, tag="rsb")
                      with nc.allow_low_precision(
                              reason="bf16 1/sum: 0.4% on normalization"):
                          nc.vector.reciprocal(rsb[:], sps[:])
                      # context matmuls overlap the ssb/reciprocal chain
                      cps = ps_c.tile([P, QCH], fp32, tag="ctxps")
                      for kb in range(nkb):
                          dd = max(kb * P - q0, 0)
                          nc.tensor.matmul(cps[:, dd:],
                                           vS[b][:, kb,
                                                 h * DH:(h + 1) * DH],
                                           wtile[:, kb, dd:],
                                           start=(kb == 0),
                                           stop=(kb == nkb - 1))
                      # broadcast 1/sum across partitions via K=1 matmul
                      rpsv = ps_v.tile([P, 512], fp32, tag="projv")
                      rps = rpsv[:, :QCH]
                      nc.tensor.matmul(rps[:], ones_row[:], rsb[:],
                                       start=True, stop=True,
                                       skip_group_check=True)
                      rbc = tmp.tile([P, QCH], fp32, tag="rbc_sb")
                      nc.scalar.activation(rbc[:], rps[:], COPY)
                      nc.vector.tensor_mul(ctxT[:, h, :], cps[:], rbc[:])

                  # partial output projection for this q-chunk
                  for ts_ in range(QCH // P):
                      ot = outp.tile([P, H], bf16, tag="otile")
                      for hc in range(H // 512):
                          ps = ps_qk.tile([P, 512], fp32, tag="qk")
                          for h in range(HPC):
                              nc.tensor.matmul(
                                  ps[:], ctxT[:, h, ts_ * P:(ts_ + 1) * P],
                                  wo_t[:, h, hc * 512:(hc + 1) * 512],
                                  start=(h == 0), stop=(h == HPC - 1))
                          if hc % 2 == 0:
                              nc.scalar.activation(
                                  ot[:, hc * 512:(hc + 1) * 512], ps[:], COPY)
                          else:
                              nc.vector.tensor_copy(
                                  ot[:, hc * 512:(hc + 1) * 512], ps[:])
                      r0 = b * S + q0 + ts_ * P
                      if b == N - 1 and qc == n_qch - 1 and ts_ == 3:
                          nc.sync.dma_start(out[r0:r0 + P, :H // 2],
                                            ot[:, :H // 2])
                          nc.scalar.dma_start(out[r0:r0 + P, H // 2:],
                                              ot[:, H // 2:])
                      else:
                          nc.sync.dma_start(out[r0:r0 + P, :], ot[:])

    nc.compile()
    return nc


@lru_cache(maxsize=2)
def _get_nc(repeat=1):
    return _build_nc(repeat)


def _host_prep(X, position_ids, Wq, Wk, Wv, Wo):
    bf = ml_dtypes.bfloat16
    # xtc[c, p, k, t] = X.T[k*128+p, c*512+t]
    xtb = np.ascontiguousarray(X.reshape(T, H).T).astype(bf)   # [H, T]
    xtc = np.ascontiguousarray(
        xtb.reshape(KI, P, T // TCH, TCH).transpose(2, 1, 0, 3))

    pos = np.asarray(position_ids).astype(np.float64)
    j = np.arange(HALF, dtype=np.float64)
    theta = 1.0 / (10000.0 ** (2.0 * j / DH))
    ang = pos[:, None] * theta[None, :]            # [S, half]
    cosv = np.cos(ang).T.astype(np.float32)        # [half, S]
    sinv = np.sin(ang).T.astype(np.float32)
    cos2 = np.concatenate([cosv, cosv], axis=0)    # [128, S]

    trilm = (np.arange(P)[:, None] <= np.arange(P)[None, :]).astype(bf)

    in_maps = []
    for c in range(N_CORES):
        r0, r1 = c * HPC * DH, (c + 1) * HPC * DH
        in_maps.append({
            "xtc": xtc,
            "wqt": np.ascontiguousarray(
                Wq[r0:r1, :].T.reshape(KI, P, HPC * DH)
                .transpose(1, 0, 2)).astype(bf),
            "wkt": np.ascontiguousarray(
                Wk[r0:r1, :].T.reshape(KI, P, HPC * DH)
                .transpose(1, 0, 2)).astype(bf),
            "wvt": np.ascontiguousarray(
                Wv[r0:r1, :].T.reshape(KI, P, HPC * DH)
                .transpose(1, 0, 2)).astype(bf),
            "wot": np.ascontiguousarray(
                Wo[:, r0:r1].T.reshape(HPC, DH, H)
                .transpose(1, 0, 2)).astype(bf),
            "cos2": cos2, "sinp": sinv, "tril": trilm,
        })
    return in_maps


def run_once(in_maps, repeat=1):
    from concourse.bass_utils import run_bass_kernel_spmd
    nc = _get_nc(repeat)
    return run_bass_kernel_spmd(nc, in_maps, list(range(N_CORES)))


def kernel(X, position_ids, mask, Wq, Wk, Wv, Wo, bo, _trace=False):
    from concourse.bass_utils import run_bass_kernel_spmd

    X = np.asarray(X, dtype=np.float32)
    in_maps = _host_prep(X, position_ids,
                         np.asarray(Wq, dtype=np.float32),
                         np.asarray(Wk, dtype=np.float32),
                         np.asarray(Wv, dtype=np.float32),
                         np.asarray(Wo, dtype=np.float32))

    nc = _get_nc()
    res = run_bass_kernel_spmd(nc, in_maps, list(range(N_CORES)),
                               trace=_trace)
    acc = np.zeros((T, H), dtype=np.float32)
    for c in range(N_CORES):
        acc += res.results[c]["out"].astype(np.float32)
    acc += np.asarray(bo, dtype=np.float32)[None, :]
    out = acc.reshape(N, S, H)
    if _trace:
        return out, res
    return out
